# revision 9
# baseline (speedup 1.0000x reference)
"""CrossEfficientAttention on 8 Trainium2 NeuronCores.

Batch-parallel sharding: n=8 batch items, one per core (no collectives).

Per-core math (item x_q, x_k, x_v : [256, 6400]):
    q  = Wq x_q + bq ; k = Wk x_k (+bk cancels over the l-softmax) ; v = Wv x_v + bv
    k_sm = softmax_l(k); q_sm = softmax_ch/head(q)
    ctx  = k_sm @ v^T (per head, 32x32); out = Wr @ (ctx^T @ q_sm) + br + x_q

Numerics: the attention term is ~2% of the output magnitude (the residual
dominates), so the whole attention path runs in fp8(e4m3) DoubleRow on the
PE (2 MACs/cycle, contraction 256 in one pass) while the residual + biases
stay bf16/fp32. Output ships bf16 (halves out-DMA); measured rel-err ~2e-3
vs the 2e-2 gate.

Structure (one fused streaming loop + tiny boundary + output pass):
  Pass 1 (per 512-wide l-chunk): stream x_k/x_v as fp8, project into
    [l, ch] layout via DoubleRow (x chunk is the lhsT), exp(k) on ACT into
    ksm8 [128, 2, g, ch] (k = s*128+p over l), v copied to v8 with a
    ones-column pair folded in for the S_k sums; full 256x256 Gram
    accumulated with 2 DR matmuls per 256-l group (off-diagonal head
    blocks computed but discarded at the boundary - 8x fewer PE
    instructions than per-head 32x32 grams);
    interleaved q pipeline: project q (DR, weights stationary), exp(q+bq)
    into eq8 fp8, per-head sums via one DR matmul with a block-ones lhsT,
    fast reciprocal, PE broadcast back (bf16), normalize eq8 in place on
    DVE reading the PSUM broadcast directly.
  Boundary: Gram rows scaled by 1/S_k, 4 PE transposes, per-head diagonal
    blocks (+bv) extracted into bf16, A^T = ctxT^T . Wr^T, cast fp8.
  Pass 2: out = (A^T)^T . q_sm (one DR matmul per 512-chunk per ch-block)
    + br + x_q (bf16) via scalar_tensor_tensor, stream out as bf16.
"""

from contextlib import ExitStack

import ml_dtypes
import numpy as np

import concourse.bacc as bacc
import concourse.bass as bass
import concourse.tile as tile
from concourse import mybir
from concourse.bass_utils import run_bass_kernel_spmd

F32 = mybir.dt.float32
BF16 = mybir.dt.bfloat16
FP8 = mybir.dt.float8e4
EXP = mybir.ActivationFunctionType.Exp
MULT = mybir.AluOpType.mult
ADD = mybir.AluOpType.add
DR = mybir.MatmulPerfMode.DoubleRow

N_CORES = 8
N, CIN, H_IMG, W_IMG = 8, 256, 80, 80
L = H_IMG * W_IMG            # 6400
HEADS = 8
NG = 25                      # 256-l gram groups
LW = 512                     # streaming l tile width
NLW = (L + LW - 1) // LW     # 13 (12x512 + 1x256)

C8_COLS = 912                # fp8 pack per ktile: wq|wk|wv|bones|pad|ind8 (16-aligned for dual-fp8 LDW)
CB_COLS = 512                # bf16 pack: wr
CF_COLS = 135                # f32 pack: bq|bv|br|ident|m2 (exp shift)


def _emit(tc: tile.TileContext, ins: dict, out_ap: bass.AP):
    nc = tc.nc
    es = ExitStack()

    # ---------------- persistent consts (3 DMAs) ----------------
    cpool = es.enter_context(tc.tile_pool(name="consts", bufs=1))
    c8 = cpool.tile([128, 2, C8_COLS], FP8, name="c8")
    cb = cpool.tile([128, CB_COLS], BF16, name="cb")
    cf = cpool.tile([128, CF_COLS], F32, name="cf")
    WQ8 = c8[:, :, 0:256]
    WK8 = c8[:, :, 256:512]
    WV8 = c8[:, :, 512:768]
    BONES8 = c8[:, :, 768:776]
    IND8F = c8[0:8, :, 784:912]
    WR = cb[:, 0:512]
    BQ = cf[:, 0:2]
    BV = cf[:, 2:4]
    BR = cf[:, 4:6]
    IDENT = cf[:, 6:134]
    M2 = cf[:, 134:135]

    at8 = cpool.tile([128, 2, 256], FP8, name="at8")
    eq8 = cpool.tile([128, 2, L], FP8, name="eq8")
    xq8_sb = cpool.tile([128, 2, L], FP8, name="xq8")
    xqb_sb = cpool.tile([128, 2, L], BF16, name="xqb")

    xq8_ap, xqb_ap, xk_ap, xv_ap = ins["xq8"], ins["xqb"], ins["xk8"], ins["xv8"]

    # ================= pools =================
    es_a = ExitStack()
    kvpool = es_a.enter_context(tc.tile_pool(name="kv", bufs=2))
    rtpool = es_a.enter_context(tc.tile_pool(name="rt", bufs=4))
    bpool = es_a.enter_context(tc.tile_pool(name="bnd", bufs=1))
    # PSUM stack (8 banks): pq(2)+ps(2) at the bottom all kernel long;
    # pass1 adds ctx(2)+pkv(2); boundary swaps pkv->bndp; phase2 swaps
    # ctx+bndp -> prb(2)+po(2).
    pools = {}
    pools["pq"] = es_a.enter_context(tc.tile_pool(name="pq", bufs=2, space="PSUM"))
    pools["ps"] = es_a.enter_context(tc.tile_pool(name="ps", bufs=2, space="PSUM"))
    es_ctx = ExitStack()
    bigpool = es_ctx.enter_context(tc.tile_pool(name="big", bufs=1))
    ctxpool = es_ctx.enter_context(tc.tile_pool(name="ctxp", bufs=1, space="PSUM"))
    es_kv = ExitStack()
    pkv = es_kv.enter_context(tc.tile_pool(name="pkv", bufs=2, space="PSUM"))

    ksm8 = bigpool.tile([128, 2, NG, 256], FP8, name="ksm8")
    v8 = bigpool.tile([128, 2, NG, 258], FP8, name="v8")
    # ones columns folded into v8 -> the Gram's 2 spare output columns are
    # the softmax sums S_k
    nc.vector.memset(v8[:, :, :, 256:258], 1.0)

    ctx_ps = [ctxpool.tile([128, 258], F32, name=f"ctx{c}") for c in range(2)]

    def gram(g):
        # full 256x256 Gram (+S_k cols); per-head blocks extracted at the
        # boundary, off-diagonal blocks discarded
        for c in range(2):
            nc.tensor.matmul(
                ctx_ps[c][:, 0:258],
                ksm8[:, :, g, 128 * c : 128 * c + 128],
                v8[:, :, g, :],
                start=(g == 0), stop=(g == NG - 1),
                perf_mode=DR,
            )

    def qwork(a):
        # q projection + exp for chunk a
        w = min(LW, L - a * LW)
        l0 = a * LW
        for c in range(2):
            pq = pools["pq"].tile([128, w], F32, name="pq")
            nc.tensor.matmul(
                pq[:], WQ8[:, :, 128 * c : 128 * c + 128],
                xq8_sb[:, :, l0 : l0 + w],
                start=True, stop=True, perf_mode=DR,
            )
            nc.scalar.activation(
                eq8[:, c, l0 : l0 + w], pq[:], EXP, bias=BQ[:, c : c + 1]
            )

    rtb_tiles = {}

    def qsum(a):
        # head sums + reciprocal for chunk a (bcast/normalize run a chunk later)
        w = min(LW, L - a * LW)
        l0 = a * LW
        psS = pools["ps"].tile([8, w], F32, name="psS")
        nc.tensor.matmul(
            psS[:], BONES8, eq8[:, :, l0 : l0 + w],
            start=True, stop=True, perf_mode=DR,
        )
        rt = rtpool.tile([8, w], F32, name="rt")
        rt8 = rtpool.tile([8, w], FP8, name="rt8")
        nc.vector.reciprocal_approx_fast(rt[:], psS[:])
        nc.gpsimd.tensor_copy(rt8[:], rt[:])
        rtb_tiles[a] = rt8

    def qnorm(a, c):
        # broadcast 1/S to all head partitions (PE bf16), then normalize
        # eq8 in place on DVE reading the PSUM broadcast directly
        w = min(LW, L - a * LW)
        l0 = a * LW
        rt8 = rtb_tiles[a] if c == 0 else rtb_tiles.pop(a)
        prb = pools["prb"].tile([128, w], F32, name="prb")
        nc.tensor.matmul(prb[:], IND8F[:, c, :], rt8[:],
                         start=True, stop=True)
        nc.vector.tensor_tensor(
            eq8[:, c, l0 : l0 + w], eq8[:, c, l0 : l0 + w], prb[:], op=MULT
        )

    # ================= pass 1: k/v proj + Gram with interleaved q =================
    xk_t = xv_t = None
    consts_loaded = False
    for a in range(NLW):
        w = min(LW, L - a * LW)
        l0 = a * LW
        if a % 4 == 0:
            # 2048-wide loads (4 chunks worth) to amortize per-DMA dispatch;
            # the first batch's k/v loads are split so chunk-0 compute only
            # waits on a 512-wide slice
            wd = min(4 * LW, L - l0)
            if not consts_loaded:
                consts_loaded = True
                nc.sync.dma_start(c8[:], ins["c8"][:])
                nc.scalar.dma_start(cb[:], ins["cb"][:])
                nc.scalar.dma_start(cf[:], ins["cf"][:])
            xk_t = kvpool.tile([128, 2, wd], FP8, name="xk")
            xv_t = kvpool.tile([128, 2, wd], FP8, name="xv")
            for s in range(2):
                if a == 0:
                    nc.sync.dma_start(xk_t[:, s, 0:512], xk_ap[128 * s : 128 * (s + 1), 0:512])
                    nc.sync.dma_start(xv_t[:, s, 0:512], xv_ap[128 * s : 128 * (s + 1), 0:512])
                    nc.sync.dma_start(xk_t[:, s, 512:wd], xk_ap[128 * s : 128 * (s + 1), 512:wd])
                    nc.sync.dma_start(xv_t[:, s, 512:wd], xv_ap[128 * s : 128 * (s + 1), 512:wd])
                else:
                    nc.sync.dma_start(xk_t[:, s, :], xk_ap[128 * s : 128 * (s + 1), l0 : l0 + wd])
                    nc.sync.dma_start(xv_t[:, s, :], xv_ap[128 * s : 128 * (s + 1), l0 : l0 + wd])
            for s in range(2):
                nc.scalar.dma_start(
                    xq8_sb[:, s, l0 : l0 + wd], xq8_ap[128 * s : 128 * (s + 1), l0 : l0 + wd]
                )
                nc.scalar.dma_start(
                    xqb_sb[:, s, l0 : l0 + wd], xqb_ap[128 * s : 128 * (s + 1), l0 : l0 + wd]
                )
        off = 512 * (a % 4)
        for jj in range(w // 256):  # one 256-l gram group per psum tile
            g = 2 * a + jj
            pk = pkv.tile([128, 512], F32, name="pkv")
            pv = pkv.tile([128, 512], F32, name="pkv")
            for j in range(2):  # j = s of the group (128-l block)
                o = off + 256 * jj + 128 * j
                nc.tensor.matmul(
                    pk[:, 256 * j : 256 * j + 256],
                    xk_t[:, :, o : o + 128], WK8,
                    start=True, stop=True, perf_mode=DR,
                )
                nc.tensor.matmul(
                    pv[:, 256 * j : 256 * j + 256],
                    xv_t[:, :, o : o + 128], WV8,
                    start=True, stop=True, perf_mode=DR,
                )
            nc.scalar.activation(ksm8[:, :, g, :], pk[:], EXP, bias=M2)
            if g % 4 == 1:
                nc.scalar.copy(v8[:, :, g, 0:256], pv[:])
            else:
                nc.vector.tensor_copy(v8[:, :, g, 0:256], pv[:])
            if g - 2 >= 0:
                gram(g - 2)

    for g in range(NG - 2, NG):
        gram(g)

    es_kv.close()  # release pk/pv banks

    # q pipeline ramp (chunks 0-1) overlaps the gram drain + boundary
    qwork(0)
    qwork(1)
    qsum(0)

    # ---------------- boundary: build A^T [kch, cout] (fp8) ----------------
    es_bnd = ExitStack()
    bpsum = es_bnd.enter_context(tc.tile_pool(name="bndp", bufs=2, space="PSUM"))
    rk = [bpool.tile([128, 1], F32, name=f"rk{c}") for c in range(2)]
    ctxs = [bpool.tile([128, 256], F32, name=f"ctxs{c}") for c in range(2)]
    for c in range(2):
        nc.vector.reciprocal(rk[c][:], ctx_ps[c][:, 256:257])
        nc.vector.tensor_scalar_mul(ctxs[c][:], ctx_ps[c][:, 0:256], rk[c][:])
    ctxT_ps = [bpsum.tile([128, 256], F32, name="bnd") for a in range(2)]
    for a in range(2):
        for b in range(2):
            nc.tensor.transpose(
                ctxT_ps[a][:, 128 * b : 128 * b + 128],
                ctxs[b][:, 128 * a : 128 * a + 128],
                IDENT,
            )
    ctxT_sb = [bpool.tile([128, 256], BF16, name=f"ctxTs{a}") for a in range(2)]
    for a in range(2):
        nc.vector.memset(ctxT_sb[a][:], 0.0)
    for h in range(HEADS):
        a = h // 4
        p = 32 * (h % 4)
        nc.vector.tensor_scalar_add(
            ctxT_sb[a][p : p + 32, 32 * h : 32 * h + 32],
            ctxT_ps[a][p : p + 32, 32 * h : 32 * h + 32],
            BV[p : p + 32, a : a + 1],
        )
    for g in range(2):
        at_ps = bpsum.tile([128, 256], F32, name="bnd")
        for a in range(2):
            nc.tensor.matmul(
                at_ps[:],
                ctxT_sb[a][:, 128 * g : 128 * g + 128],
                WR[:, 256 * a : 256 * a + 256],
                start=(a == 0), stop=(a == 1),
            )
        nc.scalar.copy(at8[:, g, :], at_ps[:])
    es_bnd.close()
    es_ctx.close()

    # ========= phase 2: q pipeline + pass-2 output, per 512-chunk =========
    es_c = ExitStack()
    opool = es_c.enter_context(tc.tile_pool(name="op", bufs=3))
    pools["prb"] = es_c.enter_context(tc.tile_pool(name="prb", bufs=2, space="PSUM"))
    po_pool = es_c.enter_context(tc.tile_pool(name="po", bufs=2, space="PSUM"))

    def pass2(a):
        w = min(LW, L - a * LW)
        l0 = a * LW
        for c in range(2):
            ob = opool.tile([128, w], BF16, name="ob")
            po = po_pool.tile([128, w], F32, name="po")
            nc.tensor.matmul(
                po[:], at8[:, :, 128 * c : 128 * c + 128],
                eq8[:, :, l0 : l0 + w],
                start=True, stop=True, perf_mode=DR,
            )
            if (a + c) % 2 == 0:
                nc.vector.scalar_tensor_tensor(
                    ob[:], po[:], BR[:, c : c + 1],
                    xqb_sb[:, c, l0 : l0 + w], op0=ADD, op1=ADD,
                )
            else:
                nc.scalar.activation(
                    ob[:], po[:],
                    mybir.ActivationFunctionType.Identity, bias=BR[:, c : c + 1],
                )
                nc.gpsimd.tensor_tensor(
                    ob[:], ob[:], xqb_sb[:, c, l0 : l0 + w], op=ADD,
                )
            nc.sync.dma_start(out_ap[128 * c : 128 * c + 128, l0 : l0 + w], ob[:])

    for t in range(2, NLW + 3):
        if t < NLW:
            qwork(t)
        if t - 1 < NLW:
            qsum(t - 1)
        if 0 <= t - 2 < NLW:
            qnorm(t - 2, 0)
            qnorm(t - 2, 1)
        if 0 <= t - 3 < NLW:
            pass2(t - 3)
    es_c.close()
    es_a.close()
    es.close()


def _build_consts(Wq, bq, Wk, bk, Wv, bv, Wr, br):
    bf = ml_dtypes.bfloat16
    f8 = ml_dtypes.float8_e4m3

    def packT8(W):  # [cout, cin] -> [128, 2, 256]: [p, s, o] = W[o, s*128+p]
        t = np.ascontiguousarray(np.asarray(W, np.float32).T)  # [cin, cout]
        return np.stack([t[0:128, :], t[128:256, :]], axis=1)

    ch = np.arange(256)
    bones_full = (ch[:, None] // 32 == np.arange(8)[None, :]).astype(np.float32)  # [256, 8]
    bones8 = np.stack([bones_full[0:128, :], bones_full[128:256, :]], axis=1)  # [128,2,8]
    ind8f = np.zeros((128, 2, 128), np.float32)   # [h, c, j] = head indicator
    for c in range(2):
        for j in range(128):
            ind8f[(c * 128 + j) // 32, c, j] = 1.0
    c8 = np.concatenate(
        [packT8(Wq), packT8(Wk), packT8(Wv), bones8,
         np.zeros((128, 2, 8), np.float32), ind8f], axis=2
    ).astype(f8).reshape(128, 2 * C8_COLS)

    def packT(Wt):  # [cout, cin] -> [128, 512], col block k = W.T[128k:128k+128, :]
        t = np.ascontiguousarray(np.asarray(Wt, np.float32).T)
        return np.concatenate([t[0:128, :], t[128:256, :]], axis=1)

    cb = packT(Wr).astype(bf)
    assert cb.shape == (128, CB_COLS), cb.shape

    def two(v):
        return np.stack([v[0:128], v[128:256]], axis=1).astype(np.float32)

    cf = np.concatenate(
        [two(np.asarray(bq)) - 2.0, two(np.asarray(bv)), two(np.asarray(br)),
         np.eye(128, dtype=np.float32),
         np.full((128, 1), -2.0, np.float32)], axis=1
    ).astype(np.float32)
    assert cf.shape == (128, CF_COLS), cf.shape
    return {"c8": c8, "cb": cb, "cf": cf}


_NC = None


def _build():
    nc = bacc.Bacc("TRN2", target_bir_lowering=False)
    ins = {}
    ins["xq8"] = nc.dram_tensor("xq8", [CIN, L], FP8, kind="ExternalInput").ap()
    ins["xqb"] = nc.dram_tensor("xqb", [CIN, L], BF16, kind="ExternalInput").ap()
    ins["xk8"] = nc.dram_tensor("xk8", [CIN, L], FP8, kind="ExternalInput").ap()
    ins["xv8"] = nc.dram_tensor("xv8", [CIN, L], FP8, kind="ExternalInput").ap()
    ins["c8"] = nc.dram_tensor("c8", [128, 2 * C8_COLS], FP8, kind="ExternalInput").ap()
    ins["cb"] = nc.dram_tensor("cb", [128, CB_COLS], BF16, kind="ExternalInput").ap()
    ins["cf"] = nc.dram_tensor("cf", [128, CF_COLS], F32, kind="ExternalInput").ap()
    out_ap = nc.dram_tensor("out", [CIN, L], BF16, kind="ExternalOutput").ap()
    with tile.TileContext(nc) as tc:
        _emit(tc, ins, out_ap)
    nc.compile()
    return nc


def get_nc():
    global _NC
    if _NC is None:
        _NC = _build()
    return _NC


def make_in_maps(inputs):
    bf = ml_dtypes.bfloat16
    f8 = ml_dtypes.float8_e4m3
    consts = _build_consts(
        inputs["Wq"], inputs["bq"], inputs["Wk"], inputs["bk"],
        inputs["Wv"], inputs["bv"], inputs["Wr"], inputs["br"],
    )
    qf = np.ascontiguousarray(np.asarray(inputs["query_feature"], np.float32)).reshape(N, CIN, L)
    kf = np.asarray(inputs["key_feature"], np.float32).reshape(N, CIN, L)
    vf = np.asarray(inputs["value_feature"], np.float32).reshape(N, CIN, L)
    return [
        {"xq8": np.ascontiguousarray(qf[i].astype(f8)),
         "xqb": np.ascontiguousarray(qf[i].astype(bf)),
         "xk8": np.ascontiguousarray(kf[i].astype(f8)),
         "xv8": np.ascontiguousarray(vf[i].astype(f8)),
         **consts}
        for i in range(N_CORES)
    ]


def kernel(query_feature, key_feature, value_feature,
           Wq, bq, Wk, bk, Wv, bv, Wr, br):
    nc = get_nc()
    in_maps = make_in_maps(dict(
        query_feature=query_feature, key_feature=key_feature,
        value_feature=value_feature, Wq=Wq, bq=bq, Wk=Wk, bk=bk,
        Wv=Wv, bv=bv, Wr=Wr, br=br,
    ))
    res = run_bass_kernel_spmd(nc, in_maps, core_ids=list(range(N_CORES)))
    out = np.stack([res.results[i]["out"].astype(np.float32) for i in range(N_CORES)])
    return out.reshape(N, CIN, H_IMG, W_IMG)


# revision 10
# speedup vs baseline: 1.0427x; 1.0427x over previous
"""CrossEfficientAttention on 8 Trainium2 NeuronCores.

Batch-parallel sharding: n=8 batch items, one per core (no collectives).

Per-core math (item x_q, x_k, x_v : [256, 6400]):
    q  = Wq x_q + bq ; k = Wk x_k (+bk cancels over the l-softmax) ; v = Wv x_v + bv
    k_sm = softmax_l(k); q_sm = softmax_ch/head(q)
    ctx  = k_sm @ v^T (per head, 32x32); out = Wr @ (ctx^T @ q_sm) + br + x_q

Numerics: the attention term is ~2% of the output magnitude (the residual
dominates), so the whole attention path runs in fp8(e4m3) DoubleRow on the
PE (2 MACs/cycle, contraction 256 in one pass) while the residual + biases
stay bf16/fp32. Output ships bf16 (halves out-DMA); measured rel-err ~2e-3
vs the 2e-2 gate.

Structure (one fused streaming loop + tiny boundary + output pass):
  Pass 1 (per 512-wide l-chunk): stream x_k/x_v as fp8, project into
    [l, ch] layout via DoubleRow (x chunk is the lhsT), exp(k) on ACT into
    ksm8 [128, 2, g, ch] (k = s*128+p over l), v copied to v8 with a
    ones-column pair folded in for the S_k sums; full 256x256 Gram
    accumulated with 2 DR matmuls per 256-l group (off-diagonal head
    blocks computed but discarded at the boundary - 8x fewer PE
    instructions than per-head 32x32 grams);
    interleaved q pipeline: project q (DR, weights stationary), exp(q+bq)
    into eq8 fp8, per-head sums via one DR matmul with a block-ones lhsT,
    fast reciprocal, PE broadcast back (bf16), normalize eq8 in place on
    DVE reading the PSUM broadcast directly.
  Boundary: Gram rows scaled by 1/S_k, 4 PE transposes, per-head diagonal
    blocks (+bv) extracted into bf16, A^T = ctxT^T . Wr^T, cast fp8.
  Pass 2: out = (A^T)^T . q_sm (one DR matmul per 512-chunk per ch-block)
    + br + x_q (bf16) via scalar_tensor_tensor, stream out as bf16.
"""

from contextlib import ExitStack

import ml_dtypes
import numpy as np

import concourse.bacc as bacc
import concourse.bass as bass
import concourse.tile as tile
from concourse import mybir
from concourse.bass_utils import run_bass_kernel_spmd

F32 = mybir.dt.float32
BF16 = mybir.dt.bfloat16
FP8 = mybir.dt.float8e4
EXP = mybir.ActivationFunctionType.Exp
MULT = mybir.AluOpType.mult
ADD = mybir.AluOpType.add
DR = mybir.MatmulPerfMode.DoubleRow

N_CORES = 8
N, CIN, H_IMG, W_IMG = 8, 256, 80, 80
L = H_IMG * W_IMG            # 6400
HEADS = 8
NG = 25                      # 256-l gram groups
LW = 512                     # streaming l tile width
NLW = (L + LW - 1) // LW     # 13 (12x512 + 1x256)

C8_COLS = 912                # fp8 pack per ktile: wq|wk|wv|bones|pad|ind8 (16-aligned for dual-fp8 LDW)
CB_COLS = 512                # bf16 pack: wr
CF_COLS = 135                # f32 pack: bq|bv|br|ident|m2 (exp shift)


def _emit(tc: tile.TileContext, ins: dict, out_ap: bass.AP):
    nc = tc.nc
    es = ExitStack()

    # ---------------- persistent consts (3 DMAs) ----------------
    cpool = es.enter_context(tc.tile_pool(name="consts", bufs=1))
    c8 = cpool.tile([128, 2, C8_COLS], FP8, name="c8")
    cb = cpool.tile([128, CB_COLS], BF16, name="cb")
    cf = cpool.tile([128, CF_COLS], F32, name="cf")
    WQ8 = c8[:, :, 0:256]
    WK8 = c8[:, :, 256:512]
    WV8 = c8[:, :, 512:768]
    BONES8 = c8[:, :, 768:776]
    IND8F = c8[0:8, :, 784:912]
    WR = cb[:, 0:512]
    BQ = cf[:, 0:2]
    BV = cf[:, 2:4]
    BR = cf[:, 4:6]
    IDENT = cf[:, 6:134]
    M2 = cf[:, 134:135]

    at8 = cpool.tile([128, 2, 256], FP8, name="at8")
    eq8 = cpool.tile([128, 2, L], FP8, name="eq8")
    xq8_sb = cpool.tile([128, 2, L], FP8, name="xq8")
    xqb_sb = cpool.tile([128, 2, L], BF16, name="xqb")

    xq8_ap, xqb_ap, xk_ap, xv_ap = ins["xq8"], ins["xqb"], ins["xk8"], ins["xv8"]

    # ================= pools =================
    es_a = ExitStack()
    kvpool = es_a.enter_context(tc.tile_pool(name="kv", bufs=2))
    rtpool = es_a.enter_context(tc.tile_pool(name="rt", bufs=4))
    bpool = es_a.enter_context(tc.tile_pool(name="bnd", bufs=1))
    # PSUM stack (8 banks): pq(2)+ps(2) at the bottom all kernel long;
    # pass1 adds ctx(2)+pkv(2); boundary swaps pkv->bndp; phase2 swaps
    # ctx+bndp -> prb(2)+po(2).
    pools = {}
    pools["pq"] = es_a.enter_context(tc.tile_pool(name="pq", bufs=2, space="PSUM"))
    pools["ps"] = es_a.enter_context(tc.tile_pool(name="ps", bufs=1, space="PSUM"))
    pools["prb"] = es_a.enter_context(tc.tile_pool(name="prb", bufs=1, space="PSUM"))
    es_ctx = ExitStack()
    bigpool = es_ctx.enter_context(tc.tile_pool(name="big", bufs=1))
    ctxpool = es_ctx.enter_context(tc.tile_pool(name="ctxp", bufs=1, space="PSUM"))
    es_kv = ExitStack()
    pkv = es_kv.enter_context(tc.tile_pool(name="pkv", bufs=2, space="PSUM"))

    ksm8 = bigpool.tile([128, 2, NG, 256], FP8, name="ksm8")
    v8 = bigpool.tile([128, 2, NG, 258], FP8, name="v8")
    # ones columns folded into v8 -> the Gram's 2 spare output columns are
    # the softmax sums S_k
    nc.vector.memset(v8[:, :, :, 256:258], 1.0)

    ctx_ps = [ctxpool.tile([128, 258], F32, name=f"ctx{c}") for c in range(2)]

    def gram(g):
        # full 256x256 Gram (+S_k cols); per-head blocks extracted at the
        # boundary, off-diagonal blocks discarded
        for c in range(2):
            nc.tensor.matmul(
                ctx_ps[c][:, 0:258],
                ksm8[:, :, g, 128 * c : 128 * c + 128],
                v8[:, :, g, :],
                start=(g == 0), stop=(g == NG - 1),
                perf_mode=DR,
            )

    def qwork(a):
        # q projection + exp for chunk a
        w = min(LW, L - a * LW)
        l0 = a * LW
        for c in range(2):
            pq = pools["pq"].tile([128, w], F32, name="pq")
            nc.tensor.matmul(
                pq[:], WQ8[:, :, 128 * c : 128 * c + 128],
                xq8_sb[:, :, l0 : l0 + w],
                start=True, stop=True, perf_mode=DR,
            )
            nc.scalar.activation(
                eq8[:, c, l0 : l0 + w], pq[:], EXP, bias=BQ[:, c : c + 1]
            )

    rtb_tiles = {}

    def qsum(a):
        # head sums + reciprocal for chunk a (bcast/normalize run a chunk later)
        w = min(LW, L - a * LW)
        l0 = a * LW
        psS = pools["ps"].tile([8, w], F32, name="psS")
        nc.tensor.matmul(
            psS[:], BONES8, eq8[:, :, l0 : l0 + w],
            start=True, stop=True, perf_mode=DR,
        )
        rt = rtpool.tile([8, w], F32, name="rt")
        rt8 = rtpool.tile([8, w], FP8, name="rt8")
        nc.vector.reciprocal_approx_fast(rt[:], psS[:])
        nc.gpsimd.tensor_copy(rt8[:], rt[:])
        rtb_tiles[a] = rt8

    def qstages(t):
        if 0 <= t - 2 < NLW:
            qwork(t - 2)
        if 0 <= t - 3 < NLW:
            qsum(t - 3)
        if 0 <= t - 4 < NLW:
            qnorm(t - 4, 0)
        if 0 <= t - 5 < NLW:
            qnorm(t - 5, 1)

    def qnorm(a, c):
        # broadcast 1/S to all head partitions (PE bf16), then normalize
        # eq8 in place on DVE reading the PSUM broadcast directly
        w = min(LW, L - a * LW)
        l0 = a * LW
        rt8 = rtb_tiles[a] if c == 0 else rtb_tiles.pop(a)
        prb = pools["prb"].tile([128, w], F32, name="prb")
        nc.tensor.matmul(prb[:], IND8F[:, c, :], rt8[:],
                         start=True, stop=True)
        nc.vector.tensor_tensor(
            eq8[:, c, l0 : l0 + w], eq8[:, c, l0 : l0 + w], prb[:], op=MULT
        )

    # ================= pass 1: k/v proj + Gram with interleaved q =================
    xk_t = xv_t = None
    consts_loaded = False
    for a in range(NLW):
        w = min(LW, L - a * LW)
        l0 = a * LW
        if a % 4 == 0:
            # 2048-wide loads (4 chunks worth) to amortize per-DMA dispatch;
            # the first batch's k/v loads are split so chunk-0 compute only
            # waits on a 512-wide slice
            wd = min(4 * LW, L - l0)
            if not consts_loaded:
                consts_loaded = True
                nc.sync.dma_start(c8[:], ins["c8"][:])
                nc.scalar.dma_start(cb[:], ins["cb"][:])
                nc.scalar.dma_start(cf[:], ins["cf"][:])
            xk_t = kvpool.tile([128, 2, wd], FP8, name="xk")
            xv_t = kvpool.tile([128, 2, wd], FP8, name="xv")
            for s in range(2):
                if a == 0:
                    nc.sync.dma_start(xk_t[:, s, 0:512], xk_ap[128 * s : 128 * (s + 1), 0:512])
                    nc.sync.dma_start(xv_t[:, s, 0:512], xv_ap[128 * s : 128 * (s + 1), 0:512])
                    nc.sync.dma_start(xk_t[:, s, 512:wd], xk_ap[128 * s : 128 * (s + 1), 512:wd])
                    nc.sync.dma_start(xv_t[:, s, 512:wd], xv_ap[128 * s : 128 * (s + 1), 512:wd])
                else:
                    nc.sync.dma_start(xk_t[:, s, :], xk_ap[128 * s : 128 * (s + 1), l0 : l0 + wd])
                    nc.sync.dma_start(xv_t[:, s, :], xv_ap[128 * s : 128 * (s + 1), l0 : l0 + wd])
            for s in range(2):
                nc.scalar.dma_start(
                    xq8_sb[:, s, l0 : l0 + wd], xq8_ap[128 * s : 128 * (s + 1), l0 : l0 + wd]
                )
                nc.scalar.dma_start(
                    xqb_sb[:, s, l0 : l0 + wd], xqb_ap[128 * s : 128 * (s + 1), l0 : l0 + wd]
                )
        off = 512 * (a % 4)
        for jj in range(w // 256):  # one 256-l gram group per psum tile
            g = 2 * a + jj
            pk = pkv.tile([128, 512], F32, name="pkv")
            pv = pkv.tile([128, 512], F32, name="pkv")
            for j in range(2):  # j = s of the group (128-l block)
                o = off + 256 * jj + 128 * j
                nc.tensor.matmul(
                    pk[:, 256 * j : 256 * j + 256],
                    xk_t[:, :, o : o + 128], WK8,
                    start=True, stop=True, perf_mode=DR,
                )
                nc.tensor.matmul(
                    pv[:, 256 * j : 256 * j + 256],
                    xv_t[:, :, o : o + 128], WV8,
                    start=True, stop=True, perf_mode=DR,
                )
            nc.scalar.activation(ksm8[:, :, g, :], pk[:], EXP, bias=M2)
            if g % 4 == 1:
                nc.scalar.copy(v8[:, :, g, 0:256], pv[:])
            else:
                nc.vector.tensor_copy(v8[:, :, g, 0:256], pv[:])
            if g - 2 >= 0:
                gram(g - 2)
        qstages(a)

    for g in range(NG - 2, NG):
        gram(g)

    es_kv.close()  # release pk/pv banks

    # ---------------- boundary: build A^T [kch, cout] (fp8) ----------------
    es_bnd = ExitStack()
    bpsum = es_bnd.enter_context(tc.tile_pool(name="bndp", bufs=2, space="PSUM"))
    rk = [bpool.tile([128, 1], F32, name=f"rk{c}") for c in range(2)]
    ctxs = [bpool.tile([128, 256], F32, name=f"ctxs{c}") for c in range(2)]
    for c in range(2):
        nc.vector.reciprocal(rk[c][:], ctx_ps[c][:, 256:257])
        nc.vector.tensor_scalar_mul(ctxs[c][:], ctx_ps[c][:, 0:256], rk[c][:])
    ctxT_ps = [bpsum.tile([128, 256], F32, name="bnd") for a in range(2)]
    for a in range(2):
        for b in range(2):
            nc.tensor.transpose(
                ctxT_ps[a][:, 128 * b : 128 * b + 128],
                ctxs[b][:, 128 * a : 128 * a + 128],
                IDENT,
            )
    ctxT_sb = [bpool.tile([128, 256], BF16, name=f"ctxTs{a}") for a in range(2)]
    for a in range(2):
        nc.vector.memset(ctxT_sb[a][:], 0.0)
    for h in range(HEADS):
        a = h // 4
        p = 32 * (h % 4)
        nc.vector.tensor_scalar_add(
            ctxT_sb[a][p : p + 32, 32 * h : 32 * h + 32],
            ctxT_ps[a][p : p + 32, 32 * h : 32 * h + 32],
            BV[p : p + 32, a : a + 1],
        )
    for g in range(2):
        at_ps = bpsum.tile([128, 256], F32, name="bnd")
        for a in range(2):
            nc.tensor.matmul(
                at_ps[:],
                ctxT_sb[a][:, 128 * g : 128 * g + 128],
                WR[:, 256 * a : 256 * a + 256],
                start=(a == 0), stop=(a == 1),
            )
        nc.scalar.copy(at8[:, g, :], at_ps[:])
    es_bnd.close()
    es_ctx.close()

    # ========= tail: remaining q stages interleaved with pass-2 output =========
    es_c = ExitStack()
    opool = es_c.enter_context(tc.tile_pool(name="op", bufs=3))
    po_pool = es_c.enter_context(tc.tile_pool(name="po", bufs=4, space="PSUM"))

    def pass2(a):
        wd = min(2 * LW, L - a * LW)
        ld = a * LW
        for c in range(2):
            ob = opool.tile([128, wd], BF16, name="ob")
            for hi, half in enumerate(range(0, wd, LW)):
                w = min(LW, wd - half)
                l0 = ld + half
                po = po_pool.tile([128, w], F32, name="po")
                nc.tensor.matmul(
                    po[:], at8[:, :, 128 * c : 128 * c + 128],
                    eq8[:, :, l0 : l0 + w],
                    start=True, stop=True, perf_mode=DR,
                )
                if (2 * c + hi) % 2 == 0:
                    nc.vector.scalar_tensor_tensor(
                        ob[:, half : half + w], po[:], BR[:, c : c + 1],
                        xqb_sb[:, c, l0 : l0 + w], op0=ADD, op1=ADD,
                    )
                else:
                    nc.scalar.activation(
                        ob[:, half : half + w], po[:],
                        mybir.ActivationFunctionType.Identity, bias=BR[:, c : c + 1],
                    )
                    nc.gpsimd.tensor_tensor(
                        ob[:, half : half + w], ob[:, half : half + w],
                        xqb_sb[:, c, l0 : l0 + w], op=ADD,
                    )
            nc.sync.dma_start(out_ap[128 * c : 128 * c + 128, ld : ld + wd], ob[:])

    p2 = 0  # next pass-2 pair start chunk
    for t in range(NLW, NLW + 6):
        qstages(t)
        # pair (p2, p2+1) ready once qnorm(p2+1, 1) has been emitted (t-5)
        while p2 < NLW and min(p2 + 1, NLW - 1) <= t - 5:
            pass2(p2)
            p2 += 2
    es_c.close()
    es_a.close()
    es.close()


def _build_consts(Wq, bq, Wk, bk, Wv, bv, Wr, br):
    bf = ml_dtypes.bfloat16
    f8 = ml_dtypes.float8_e4m3

    def packT8(W):  # [cout, cin] -> [128, 2, 256]: [p, s, o] = W[o, s*128+p]
        t = np.ascontiguousarray(np.asarray(W, np.float32).T)  # [cin, cout]
        return np.stack([t[0:128, :], t[128:256, :]], axis=1)

    ch = np.arange(256)
    bones_full = (ch[:, None] // 32 == np.arange(8)[None, :]).astype(np.float32)  # [256, 8]
    bones8 = np.stack([bones_full[0:128, :], bones_full[128:256, :]], axis=1)  # [128,2,8]
    ind8f = np.zeros((128, 2, 128), np.float32)   # [h, c, j] = head indicator
    for c in range(2):
        for j in range(128):
            ind8f[(c * 128 + j) // 32, c, j] = 1.0
    c8 = np.concatenate(
        [packT8(Wq), packT8(Wk), packT8(Wv), bones8,
         np.zeros((128, 2, 8), np.float32), ind8f], axis=2
    ).astype(f8).reshape(128, 2 * C8_COLS)

    def packT(Wt):  # [cout, cin] -> [128, 512], col block k = W.T[128k:128k+128, :]
        t = np.ascontiguousarray(np.asarray(Wt, np.float32).T)
        return np.concatenate([t[0:128, :], t[128:256, :]], axis=1)

    cb = packT(Wr).astype(bf)
    assert cb.shape == (128, CB_COLS), cb.shape

    def two(v):
        return np.stack([v[0:128], v[128:256]], axis=1).astype(np.float32)

    cf = np.concatenate(
        [two(np.asarray(bq)) - 2.0, two(np.asarray(bv)), two(np.asarray(br)),
         np.eye(128, dtype=np.float32),
         np.full((128, 1), -2.0, np.float32)], axis=1
    ).astype(np.float32)
    assert cf.shape == (128, CF_COLS), cf.shape
    return {"c8": c8, "cb": cb, "cf": cf}


_NC = None


def _build():
    nc = bacc.Bacc("TRN2", target_bir_lowering=False)
    ins = {}
    ins["xq8"] = nc.dram_tensor("xq8", [CIN, L], FP8, kind="ExternalInput").ap()
    ins["xqb"] = nc.dram_tensor("xqb", [CIN, L], BF16, kind="ExternalInput").ap()
    ins["xk8"] = nc.dram_tensor("xk8", [CIN, L], FP8, kind="ExternalInput").ap()
    ins["xv8"] = nc.dram_tensor("xv8", [CIN, L], FP8, kind="ExternalInput").ap()
    ins["c8"] = nc.dram_tensor("c8", [128, 2 * C8_COLS], FP8, kind="ExternalInput").ap()
    ins["cb"] = nc.dram_tensor("cb", [128, CB_COLS], BF16, kind="ExternalInput").ap()
    ins["cf"] = nc.dram_tensor("cf", [128, CF_COLS], F32, kind="ExternalInput").ap()
    out_ap = nc.dram_tensor("out", [CIN, L], BF16, kind="ExternalOutput").ap()
    with tile.TileContext(nc) as tc:
        _emit(tc, ins, out_ap)
    nc.compile()
    return nc


def get_nc():
    global _NC
    if _NC is None:
        _NC = _build()
    return _NC


def make_in_maps(inputs):
    bf = ml_dtypes.bfloat16
    f8 = ml_dtypes.float8_e4m3
    consts = _build_consts(
        inputs["Wq"], inputs["bq"], inputs["Wk"], inputs["bk"],
        inputs["Wv"], inputs["bv"], inputs["Wr"], inputs["br"],
    )
    qf = np.ascontiguousarray(np.asarray(inputs["query_feature"], np.float32)).reshape(N, CIN, L)
    kf = np.asarray(inputs["key_feature"], np.float32).reshape(N, CIN, L)
    vf = np.asarray(inputs["value_feature"], np.float32).reshape(N, CIN, L)
    return [
        {"xq8": np.ascontiguousarray(qf[i].astype(f8)),
         "xqb": np.ascontiguousarray(qf[i].astype(bf)),
         "xk8": np.ascontiguousarray(kf[i].astype(f8)),
         "xv8": np.ascontiguousarray(vf[i].astype(f8)),
         **consts}
        for i in range(N_CORES)
    ]


def kernel(query_feature, key_feature, value_feature,
           Wq, bq, Wk, bk, Wv, bv, Wr, br):
    nc = get_nc()
    in_maps = make_in_maps(dict(
        query_feature=query_feature, key_feature=key_feature,
        value_feature=value_feature, Wq=Wq, bq=bq, Wk=Wk, bk=bk,
        Wv=Wv, bv=bv, Wr=Wr, br=br,
    ))
    res = run_bass_kernel_spmd(nc, in_maps, core_ids=list(range(N_CORES)))
    out = np.stack([res.results[i]["out"].astype(np.float32) for i in range(N_CORES)])
    return out.reshape(N, CIN, H_IMG, W_IMG)


# revision 11
# speedup vs baseline: 1.1140x; 1.0684x over previous
"""CrossEfficientAttention on 8 Trainium2 NeuronCores.

Batch-parallel sharding: n=8 batch items, one per core (no collectives).

Per-core math (item x_q, x_k, x_v : [256, 6400]):
    q  = Wq x_q + bq ; k = Wk x_k (+bk cancels over the l-softmax) ; v = Wv x_v + bv
    k_sm = softmax_l(k); q_sm = softmax_ch/head(q)
    ctx  = k_sm @ v^T (per head, 32x32); out = Wr @ (ctx^T @ q_sm) + br + x_q

Numerics: the attention term is ~2% of the output magnitude (the residual
dominates), so the whole attention path runs in fp8(e4m3) DoubleRow on the
PE (2 MACs/cycle, contraction 256 in one pass) while the residual + biases
stay bf16/fp32. Output ships bf16 (halves out-DMA); measured rel-err ~2e-3
vs the 2e-2 gate.

Structure (one fused streaming loop + tiny boundary + output pass):
  Pass 1 (per 512-wide l-chunk): stream x_k/x_v as fp8, project into
    [l, ch] layout via DoubleRow (x chunk is the lhsT), exp(k) on ACT into
    ksm8 [128, 2, g, ch] (k = s*128+p over l), v copied to v8 with a
    ones-column pair folded in for the S_k sums; full 256x256 Gram
    accumulated with 2 DR matmuls per 256-l group (off-diagonal head
    blocks computed but discarded at the boundary - 8x fewer PE
    instructions than per-head 32x32 grams);
    interleaved q pipeline: project q (DR, weights stationary), exp(q+bq)
    into eq8 fp8, per-head sums via one DR matmul with a block-ones lhsT,
    fast reciprocal, PE broadcast back (bf16), normalize eq8 in place on
    DVE reading the PSUM broadcast directly.
  Boundary: Gram rows scaled by 1/S_k, 4 PE transposes, per-head diagonal
    blocks (+bv) extracted into bf16, A^T = ctxT^T . Wr^T, cast fp8.
  Pass 2: out = (A^T)^T . q_sm (one DR matmul per 512-chunk per ch-block)
    + br + x_q (bf16) via scalar_tensor_tensor, stream out as bf16.
"""

from contextlib import ExitStack

import ml_dtypes
import numpy as np

import concourse.bacc as bacc
import concourse.bass as bass
import concourse.tile as tile
from concourse import mybir
from concourse.bass_utils import run_bass_kernel_spmd

F32 = mybir.dt.float32
BF16 = mybir.dt.bfloat16
FP8 = mybir.dt.float8e4
EXP = mybir.ActivationFunctionType.Exp
MULT = mybir.AluOpType.mult
ADD = mybir.AluOpType.add
DR = mybir.MatmulPerfMode.DoubleRow

N_CORES = 8
N, CIN, H_IMG, W_IMG = 8, 256, 80, 80
L = H_IMG * W_IMG            # 6400
HEADS = 8
NG = 25                      # 256-l gram groups
LW = 512                     # streaming l tile width
NLW = (L + LW - 1) // LW     # 13 (12x512 + 1x256)

C8_COLS = 912                # fp8 pack per ktile: wq|wk|wv|bones|pad|ind8 (16-aligned for dual-fp8 LDW)
CB_COLS = 512                # bf16 pack: wr
CF_COLS = 135                # f32 pack: bq|bv|br|ident|m2 (exp shift)


def _emit(tc: tile.TileContext, ins: dict, out_ap: bass.AP):
    nc = tc.nc
    es = ExitStack()

    # ---------------- persistent consts (3 DMAs) ----------------
    cpool = es.enter_context(tc.tile_pool(name="consts", bufs=1))
    c8 = cpool.tile([128, 2, C8_COLS], FP8, name="c8")
    cb = cpool.tile([128, CB_COLS], BF16, name="cb")
    cf = cpool.tile([128, CF_COLS], F32, name="cf")
    WQ8 = c8[:, :, 0:256]
    WK8 = c8[:, :, 256:512]
    WV8 = c8[:, :, 512:768]
    BONES8 = c8[:, :, 768:776]
    IND8F = c8[0:8, :, 784:912]
    WR = cb[:, 0:512]
    BQ = cf[:, 0:2]
    BV = cf[:, 2:4]
    BR = cf[:, 4:6]
    IDENT = cf[:, 6:134]
    M2 = cf[:, 134:135]

    at8 = cpool.tile([128, 2, 256], FP8, name="at8")
    eq8 = cpool.tile([128, 2, L], FP8, name="eq8")
    xq8_sb = cpool.tile([128, 2, L], FP8, name="xq8")
    xqb_sb = cpool.tile([128, 2, L], BF16, name="xqb")

    xq8_ap, xqb_ap, xk_ap, xv_ap = ins["xq8"], ins["xqb"], ins["xk8"], ins["xv8"]

    # ================= pools =================
    es_a = ExitStack()
    kvpool = es_a.enter_context(tc.tile_pool(name="kv", bufs=2))
    rtpool = es_a.enter_context(tc.tile_pool(name="rt", bufs=4))
    bpool = es_a.enter_context(tc.tile_pool(name="bnd", bufs=1))
    # PSUM stack (8 banks): pq(2)+ps(2) at the bottom all kernel long;
    # pass1 adds ctx(2)+pkv(2); boundary swaps pkv->bndp; phase2 swaps
    # ctx+bndp -> prb(2)+po(2).
    pools = {}
    pools["pq"] = es_a.enter_context(tc.tile_pool(name="pq", bufs=2, space="PSUM"))
    pools["ps"] = es_a.enter_context(tc.tile_pool(name="ps", bufs=1, space="PSUM"))
    pools["prb"] = es_a.enter_context(tc.tile_pool(name="prb", bufs=1, space="PSUM"))
    es_ctx = ExitStack()
    bigpool = es_ctx.enter_context(tc.tile_pool(name="big", bufs=1))
    ctxpool = es_ctx.enter_context(tc.tile_pool(name="ctxp", bufs=1, space="PSUM"))
    es_kv = ExitStack()
    pkv = es_kv.enter_context(tc.tile_pool(name="pkv", bufs=2, space="PSUM"))

    ksm8 = bigpool.tile([128, 2, NG, 256], FP8, name="ksm8")
    v8 = bigpool.tile([128, 2, NG, 258], FP8, name="v8")
    # ones columns folded into v8 -> the Gram's 2 spare output columns are
    # the softmax sums S_k
    nc.vector.memset(v8[:, :, :, 256:258], 1.0)

    ctx_ps = [ctxpool.tile([128, 258], F32, name=f"ctx{c}") for c in range(2)]

    def gram(g):
        # full 256x256 Gram (+S_k cols); per-head blocks extracted at the
        # boundary, off-diagonal blocks discarded
        for c in range(2):
            nc.tensor.matmul(
                ctx_ps[c][:, 0:258],
                ksm8[:, :, g, 128 * c : 128 * c + 128],
                v8[:, :, g, :],
                start=(g == 0), stop=(g == NG - 1),
                perf_mode=DR,
            )

    def qwork(a):
        # q projection + exp for chunk a
        w = min(LW, L - a * LW)
        l0 = a * LW
        for c in range(2):
            pq = pools["pq"].tile([128, w], F32, name="pq")
            nc.tensor.matmul(
                pq[:], WQ8[:, :, 128 * c : 128 * c + 128],
                xq8_sb[:, :, l0 : l0 + w],
                start=True, stop=True, perf_mode=DR,
            )
            nc.scalar.activation(
                eq8[:, c, l0 : l0 + w], pq[:], EXP, bias=BQ[:, c : c + 1]
            )

    rtb_tiles = {}

    def qsum(a):
        # head sums + reciprocal for chunk a (bcast/normalize run a chunk later)
        w = min(LW, L - a * LW)
        l0 = a * LW
        psS = pools["ps"].tile([8, w], F32, name="psS")
        nc.tensor.matmul(
            psS[:], BONES8, eq8[:, :, l0 : l0 + w],
            start=True, stop=True, perf_mode=DR,
        )
        rt = rtpool.tile([8, w], F32, name="rt")
        rt8 = rtpool.tile([8, w], FP8, name="rt8")
        nc.vector.reciprocal_approx_fast(rt[:], psS[:])
        nc.gpsimd.tensor_copy(rt8[:], rt[:])
        rtb_tiles[a] = rt8

    def qstages(t):
        if 0 <= t - 2 < NLW:
            qwork(t - 2)
        if 0 <= t - 3 < NLW:
            qsum(t - 3)
        if 0 <= t - 4 < NLW:
            qnorm(t - 4, 0)
        if 0 <= t - 5 < NLW:
            qnorm(t - 5, 1)

    def qnorm(a, c):
        # broadcast 1/S to all head partitions (PE bf16), then normalize
        # eq8 in place on DVE reading the PSUM broadcast directly
        w = min(LW, L - a * LW)
        l0 = a * LW
        rt8 = rtb_tiles[a] if c == 0 else rtb_tiles.pop(a)
        prb = pools["prb"].tile([128, w], F32, name="prb")
        nc.tensor.matmul(prb[:], IND8F[:, c, :], rt8[:],
                         start=True, stop=True)
        nc.vector.tensor_tensor(
            eq8[:, c, l0 : l0 + w], eq8[:, c, l0 : l0 + w], prb[:], op=MULT
        )

    # ================= pass 1: k/v proj + Gram with interleaved q =================
    xk_t = xv_t = None
    consts_loaded = False
    for a in range(NLW):
        w = min(LW, L - a * LW)
        l0 = a * LW
        if a % 4 == 0:
            # 2048-wide loads (4 chunks worth) to amortize per-DMA dispatch;
            # the first batch's k/v loads are split so chunk-0 compute only
            # waits on a 512-wide slice
            wd = min(4 * LW, L - l0)
            if not consts_loaded:
                consts_loaded = True
                nc.sync.dma_start(c8[:], ins["c8"][:])
                nc.scalar.dma_start(cb[:], ins["cb"][:])
                nc.scalar.dma_start(cf[:], ins["cf"][:])
            xk_t = kvpool.tile([128, 2, wd], FP8, name="xk")
            xv_t = kvpool.tile([128, 2, wd], FP8, name="xv")
            for s in range(2):
                if a == 0:
                    nc.sync.dma_start(xk_t[:, s, 0:512], xk_ap[128 * s : 128 * (s + 1), 0:512])
                    nc.sync.dma_start(xv_t[:, s, 0:512], xv_ap[128 * s : 128 * (s + 1), 0:512])
                    nc.sync.dma_start(xk_t[:, s, 512:wd], xk_ap[128 * s : 128 * (s + 1), 512:wd])
                    nc.sync.dma_start(xv_t[:, s, 512:wd], xv_ap[128 * s : 128 * (s + 1), 512:wd])
                else:
                    nc.sync.dma_start(xk_t[:, s, :], xk_ap[128 * s : 128 * (s + 1), l0 : l0 + wd])
                    nc.sync.dma_start(xv_t[:, s, :], xv_ap[128 * s : 128 * (s + 1), l0 : l0 + wd])
            qeng = nc.scalar if a == 0 else nc.sync
            for s in range(2):
                qeng.dma_start(
                    xq8_sb[:, s, l0 : l0 + wd], xq8_ap[128 * s : 128 * (s + 1), l0 : l0 + wd]
                )
                qeng.dma_start(
                    xqb_sb[:, s, l0 : l0 + wd], xqb_ap[128 * s : 128 * (s + 1), l0 : l0 + wd]
                )
        off = 512 * (a % 4)
        for jj in range(w // 256):  # one 256-l gram group per psum tile
            g = 2 * a + jj
            pk = pkv.tile([128, 512], F32, name="pkv")
            pv = pkv.tile([128, 512], F32, name="pkv")
            for j in range(2):  # j = s of the group (128-l block)
                o = off + 256 * jj + 128 * j
                nc.tensor.matmul(
                    pk[:, 256 * j : 256 * j + 256],
                    xk_t[:, :, o : o + 128], WK8,
                    start=True, stop=True, perf_mode=DR,
                )
                nc.tensor.matmul(
                    pv[:, 256 * j : 256 * j + 256],
                    xv_t[:, :, o : o + 128], WV8,
                    start=True, stop=True, perf_mode=DR,
                )
            nc.scalar.activation(ksm8[:, :, g, :], pk[:], EXP, bias=M2)
            if g % 4 == 1:
                nc.scalar.copy(v8[:, :, g, 0:256], pv[:])
            else:
                nc.vector.tensor_copy(v8[:, :, g, 0:256], pv[:])
            if g - 2 >= 0:
                gram(g - 2)
        qstages(a)

    for g in range(NG - 2, NG):
        gram(g)

    es_kv.close()  # release pk/pv banks

    # ---------------- boundary: build A^T [kch, cout] (fp8) ----------------
    es_bnd = ExitStack()
    bpsum = es_bnd.enter_context(tc.tile_pool(name="bndp", bufs=2, space="PSUM"))
    rk = [bpool.tile([128, 1], F32, name=f"rk{c}") for c in range(2)]
    ctxs = [bpool.tile([128, 256], F32, name=f"ctxs{c}") for c in range(2)]
    for c in range(2):
        nc.vector.reciprocal(rk[c][:], ctx_ps[c][:, 256:257])
        nc.vector.tensor_scalar_mul(ctxs[c][:], ctx_ps[c][:, 0:256], rk[c][:])
    ctxT_ps = [bpsum.tile([128, 256], F32, name="bnd") for a in range(2)]
    for a in range(2):
        for b in range(2):
            nc.tensor.transpose(
                ctxT_ps[a][:, 128 * b : 128 * b + 128],
                ctxs[b][:, 128 * a : 128 * a + 128],
                IDENT,
            )
    ctxT_sb = [bpool.tile([128, 256], BF16, name=f"ctxTs{a}") for a in range(2)]
    for a in range(2):
        nc.vector.memset(ctxT_sb[a][:], 0.0)
    for h in range(HEADS):
        a = h // 4
        p = 32 * (h % 4)
        nc.vector.tensor_scalar_add(
            ctxT_sb[a][p : p + 32, 32 * h : 32 * h + 32],
            ctxT_ps[a][p : p + 32, 32 * h : 32 * h + 32],
            BV[p : p + 32, a : a + 1],
        )
    for g in range(2):
        at_ps = bpsum.tile([128, 256], F32, name="bnd")
        for a in range(2):
            nc.tensor.matmul(
                at_ps[:],
                ctxT_sb[a][:, 128 * g : 128 * g + 128],
                WR[:, 256 * a : 256 * a + 256],
                start=(a == 0), stop=(a == 1),
            )
        nc.scalar.copy(at8[:, g, :], at_ps[:])
    es_bnd.close()
    es_ctx.close()

    # ========= tail: remaining q stages interleaved with pass-2 output =========
    es_c = ExitStack()
    opool = es_c.enter_context(tc.tile_pool(name="op", bufs=3))
    po_pool = es_c.enter_context(tc.tile_pool(name="po", bufs=4, space="PSUM"))

    def pass2(a):
        wd = min(2 * LW, L - a * LW)
        ld = a * LW
        for c in range(2):
            ob = opool.tile([128, wd], BF16, name="ob")
            for hi, half in enumerate(range(0, wd, LW)):
                w = min(LW, wd - half)
                l0 = ld + half
                po = po_pool.tile([128, w], F32, name="po")
                nc.tensor.matmul(
                    po[:], at8[:, :, 128 * c : 128 * c + 128],
                    eq8[:, :, l0 : l0 + w],
                    start=True, stop=True, perf_mode=DR,
                )
                if (2 * c + hi) % 2 == 0:
                    nc.vector.scalar_tensor_tensor(
                        ob[:, half : half + w], po[:], BR[:, c : c + 1],
                        xqb_sb[:, c, l0 : l0 + w], op0=ADD, op1=ADD,
                    )
                else:
                    nc.scalar.activation(
                        ob[:, half : half + w], po[:],
                        mybir.ActivationFunctionType.Identity, bias=BR[:, c : c + 1],
                    )
                    nc.gpsimd.tensor_tensor(
                        ob[:, half : half + w], ob[:, half : half + w],
                        xqb_sb[:, c, l0 : l0 + w], op=ADD,
                    )
            nc.sync.dma_start(out_ap[128 * c : 128 * c + 128, ld : ld + wd], ob[:])

    p2 = 0  # next pass-2 pair start chunk
    for t in range(NLW, NLW + 6):
        qstages(t)
        # pair (p2, p2+1) ready once qnorm(p2+1, 1) has been emitted (t-5)
        while p2 < NLW and min(p2 + 1, NLW - 1) <= t - 5:
            pass2(p2)
            p2 += 2
    es_c.close()
    es_a.close()
    es.close()


def _build_consts(Wq, bq, Wk, bk, Wv, bv, Wr, br):
    bf = ml_dtypes.bfloat16
    f8 = ml_dtypes.float8_e4m3

    def packT8(W):  # [cout, cin] -> [128, 2, 256]: [p, s, o] = W[o, s*128+p]
        t = np.ascontiguousarray(np.asarray(W, np.float32).T)  # [cin, cout]
        return np.stack([t[0:128, :], t[128:256, :]], axis=1)

    ch = np.arange(256)
    bones_full = (ch[:, None] // 32 == np.arange(8)[None, :]).astype(np.float32)  # [256, 8]
    bones8 = np.stack([bones_full[0:128, :], bones_full[128:256, :]], axis=1)  # [128,2,8]
    ind8f = np.zeros((128, 2, 128), np.float32)   # [h, c, j] = head indicator
    for c in range(2):
        for j in range(128):
            ind8f[(c * 128 + j) // 32, c, j] = 1.0
    c8 = np.concatenate(
        [packT8(Wq), packT8(Wk), packT8(Wv), bones8,
         np.zeros((128, 2, 8), np.float32), ind8f], axis=2
    ).astype(f8).reshape(128, 2 * C8_COLS)

    def packT(Wt):  # [cout, cin] -> [128, 512], col block k = W.T[128k:128k+128, :]
        t = np.ascontiguousarray(np.asarray(Wt, np.float32).T)
        return np.concatenate([t[0:128, :], t[128:256, :]], axis=1)

    cb = packT(Wr).astype(bf)
    assert cb.shape == (128, CB_COLS), cb.shape

    def two(v):
        return np.stack([v[0:128], v[128:256]], axis=1).astype(np.float32)

    cf = np.concatenate(
        [two(np.asarray(bq)) - 2.0, two(np.asarray(bv)), two(np.asarray(br)),
         np.eye(128, dtype=np.float32),
         np.full((128, 1), -2.0, np.float32)], axis=1
    ).astype(np.float32)
    assert cf.shape == (128, CF_COLS), cf.shape
    return {"c8": c8, "cb": cb, "cf": cf}


_NC = None


def _build():
    nc = bacc.Bacc("TRN2", target_bir_lowering=False)
    ins = {}
    ins["xq8"] = nc.dram_tensor("xq8", [CIN, L], FP8, kind="ExternalInput").ap()
    ins["xqb"] = nc.dram_tensor("xqb", [CIN, L], BF16, kind="ExternalInput").ap()
    ins["xk8"] = nc.dram_tensor("xk8", [CIN, L], FP8, kind="ExternalInput").ap()
    ins["xv8"] = nc.dram_tensor("xv8", [CIN, L], FP8, kind="ExternalInput").ap()
    ins["c8"] = nc.dram_tensor("c8", [128, 2 * C8_COLS], FP8, kind="ExternalInput").ap()
    ins["cb"] = nc.dram_tensor("cb", [128, CB_COLS], BF16, kind="ExternalInput").ap()
    ins["cf"] = nc.dram_tensor("cf", [128, CF_COLS], F32, kind="ExternalInput").ap()
    out_ap = nc.dram_tensor("out", [CIN, L], BF16, kind="ExternalOutput").ap()
    with tile.TileContext(nc) as tc:
        _emit(tc, ins, out_ap)
    nc.compile()
    return nc


def get_nc():
    global _NC
    if _NC is None:
        _NC = _build()
    return _NC


def make_in_maps(inputs):
    bf = ml_dtypes.bfloat16
    f8 = ml_dtypes.float8_e4m3
    consts = _build_consts(
        inputs["Wq"], inputs["bq"], inputs["Wk"], inputs["bk"],
        inputs["Wv"], inputs["bv"], inputs["Wr"], inputs["br"],
    )
    qf = np.ascontiguousarray(np.asarray(inputs["query_feature"], np.float32)).reshape(N, CIN, L)
    kf = np.asarray(inputs["key_feature"], np.float32).reshape(N, CIN, L)
    vf = np.asarray(inputs["value_feature"], np.float32).reshape(N, CIN, L)
    return [
        {"xq8": np.ascontiguousarray(qf[i].astype(f8)),
         "xqb": np.ascontiguousarray(qf[i].astype(bf)),
         "xk8": np.ascontiguousarray(kf[i].astype(f8)),
         "xv8": np.ascontiguousarray(vf[i].astype(f8)),
         **consts}
        for i in range(N_CORES)
    ]


def kernel(query_feature, key_feature, value_feature,
           Wq, bq, Wk, bk, Wv, bv, Wr, br):
    nc = get_nc()
    in_maps = make_in_maps(dict(
        query_feature=query_feature, key_feature=key_feature,
        value_feature=value_feature, Wq=Wq, bq=bq, Wk=Wk, bk=bk,
        Wv=Wv, bv=bv, Wr=Wr, br=br,
    ))
    res = run_bass_kernel_spmd(nc, in_maps, core_ids=list(range(N_CORES)))
    out = np.stack([res.results[i]["out"].astype(np.float32) for i in range(N_CORES)])
    return out.reshape(N, CIN, H_IMG, W_IMG)


# revision 12
# speedup vs baseline: 1.1952x; 1.0728x over previous
"""CrossEfficientAttention on 8 Trainium2 NeuronCores.

Batch-parallel sharding: n=8 batch items, one per core (no collectives).

Per-core math (item x_q, x_k, x_v : [256, 6400]):
    q  = Wq x_q + bq ; k = Wk x_k (+bk cancels over the l-softmax) ; v = Wv x_v + bv
    k_sm = softmax_l(k); q_sm = softmax_ch/head(q)
    ctx  = k_sm @ v^T (per head, 32x32); out = Wr @ (ctx^T @ q_sm) + br + x_q

Numerics: the attention term is ~2% of the output magnitude (the residual
dominates), so the whole attention path runs in fp8(e4m3) DoubleRow on the
PE (2 MACs/cycle, contraction 256 in one pass) while the residual + biases
stay bf16/fp32. Output ships bf16 (halves out-DMA); measured rel-err ~2e-3
vs the 2e-2 gate.

Structure (one fused streaming loop + tiny boundary + output pass):
  Pass 1 (per 512-wide l-chunk): stream x_k/x_v as fp8, project into
    [l, ch] layout via DoubleRow (x chunk is the lhsT), exp(k) on ACT into
    ksm8 [128, 2, g, ch] (k = s*128+p over l), v copied to v8 with a
    ones-column pair folded in for the S_k sums; full 256x256 Gram
    accumulated with 2 DR matmuls per 256-l group (off-diagonal head
    blocks computed but discarded at the boundary - 8x fewer PE
    instructions than per-head 32x32 grams);
    interleaved q pipeline: project q (DR, weights stationary), exp(q+bq)
    into eq8 fp8, per-head sums via one DR matmul with a block-ones lhsT,
    fast reciprocal, PE broadcast back (bf16), normalize eq8 in place on
    DVE reading the PSUM broadcast directly.
  Boundary: Gram rows scaled by 1/S_k, 4 PE transposes, per-head diagonal
    blocks (+bv) extracted into bf16, A^T = ctxT^T . Wr^T, cast fp8.
  Pass 2: out = (A^T)^T . q_sm (one DR matmul per 512-chunk per ch-block)
    + br + x_q (bf16) via scalar_tensor_tensor, stream out as bf16.
"""

from contextlib import ExitStack

import ml_dtypes
import numpy as np

import concourse.bacc as bacc
import concourse.bass as bass
import concourse.tile as tile
from concourse import mybir
from concourse.bass_utils import run_bass_kernel_spmd

F32 = mybir.dt.float32
BF16 = mybir.dt.bfloat16
FP8 = mybir.dt.float8e4
EXP = mybir.ActivationFunctionType.Exp
MULT = mybir.AluOpType.mult
ADD = mybir.AluOpType.add
DR = mybir.MatmulPerfMode.DoubleRow

N_CORES = 8
N, CIN, H_IMG, W_IMG = 8, 256, 80, 80
L = H_IMG * W_IMG            # 6400
HEADS = 8
NG = 25                      # 256-l gram groups
LW = 512                     # streaming l tile width
NLW = (L + LW - 1) // LW     # 13 (12x512 + 1x256)

C8_COLS = 912                # fp8 pack per ktile: wq|wk|wv|bones|pad|ind8 (16-aligned for dual-fp8 LDW)
CB_COLS = 512                # bf16 pack: wr
CF_COLS = 135                # f32 pack: bq|bv|br|ident|m2 (exp shift)


def _emit(tc: tile.TileContext, ins: dict, out_ap: bass.AP):
    nc = tc.nc
    es = ExitStack()

    # ---------------- persistent consts (3 DMAs) ----------------
    cpool = es.enter_context(tc.tile_pool(name="consts", bufs=1))
    c8 = cpool.tile([128, 2, C8_COLS], FP8, name="c8")
    cb = cpool.tile([128, CB_COLS], BF16, name="cb")
    cf = cpool.tile([128, CF_COLS], F32, name="cf")
    WQ8 = c8[:, :, 0:256]
    WK8 = c8[:, :, 256:512]
    WV8 = c8[:, :, 512:768]
    BONES8 = c8[:, :, 768:776]
    IND8F = c8[0:8, :, 784:912]
    WR = cb[:, 0:512]
    BQ = cf[:, 0:2]
    BV = cf[:, 2:4]
    BR = cf[:, 4:6]
    IDENT = cf[:, 6:134]
    M2 = cf[:, 134:135]

    at8 = cpool.tile([128, 2, 256], FP8, name="at8")
    eq8 = cpool.tile([128, 2, L], FP8, name="eq8")
    xq8_sb = cpool.tile([128, 2, L], FP8, name="xq8")
    xqb_sb = cpool.tile([128, 2, L], BF16, name="xqb")

    xq8_ap, xqb_ap, xk_ap, xv_ap = ins["xq8"], ins["xqb"], ins["xk8"], ins["xv8"]

    # ================= pools =================
    es_a = ExitStack()
    kvpool = es_a.enter_context(tc.tile_pool(name="kv", bufs=2))
    rtpool = es_a.enter_context(tc.tile_pool(name="rt", bufs=4))
    bpool = es_a.enter_context(tc.tile_pool(name="bnd", bufs=1))
    # PSUM stack (8 banks): pq(2)+ps(2) at the bottom all kernel long;
    # pass1 adds ctx(2)+pkv(2); boundary swaps pkv->bndp; phase2 swaps
    # ctx+bndp -> prb(2)+po(2).
    pools = {}
    pools["pq"] = es_a.enter_context(tc.tile_pool(name="pq", bufs=2, space="PSUM"))
    pools["ps"] = es_a.enter_context(tc.tile_pool(name="ps", bufs=1, space="PSUM"))
    pools["prb"] = es_a.enter_context(tc.tile_pool(name="prb", bufs=1, space="PSUM"))
    es_ctx = ExitStack()
    bigpool = es_ctx.enter_context(tc.tile_pool(name="big", bufs=1))
    ctxpool = es_ctx.enter_context(tc.tile_pool(name="ctxp", bufs=1, space="PSUM"))
    es_kv = ExitStack()
    pkv = es_kv.enter_context(tc.tile_pool(name="pkv", bufs=2, space="PSUM"))

    ksm8 = bigpool.tile([128, 2, NG, 256], FP8, name="ksm8")
    v8 = bigpool.tile([128, 2, NG, 258], FP8, name="v8")
    # ones columns folded into v8 -> the Gram's 2 spare output columns are
    # the softmax sums S_k
    nc.vector.memset(v8[:, :, :, 256:258], 1.0)

    ctx_ps = [ctxpool.tile([128, 258], F32, name=f"ctx{c}") for c in range(2)]

    def gram(g):
        # full 256x256 Gram (+S_k cols); per-head blocks extracted at the
        # boundary, off-diagonal blocks discarded
        for c in range(2):
            nc.tensor.matmul(
                ctx_ps[c][:, 0:258],
                ksm8[:, :, g, 128 * c : 128 * c + 128],
                v8[:, :, g, :],
                start=(g == 0), stop=(g == NG - 1),
                perf_mode=DR,
            )

    def qwork(a):
        # q projection + exp for chunk a
        w = min(LW, L - a * LW)
        l0 = a * LW
        for c in range(2):
            pq = pools["pq"].tile([128, w], F32, name="pq")
            nc.tensor.matmul(
                pq[:], WQ8[:, :, 128 * c : 128 * c + 128],
                xq8_sb[:, :, l0 : l0 + w],
                start=True, stop=True, perf_mode=DR,
            )
            nc.scalar.activation(
                eq8[:, c, l0 : l0 + w], pq[:], EXP, bias=BQ[:, c : c + 1]
            )

    rtb_tiles = {}

    def qsum(a):
        # head sums + reciprocal for chunk a (bcast/normalize run a chunk later)
        w = min(LW, L - a * LW)
        l0 = a * LW
        psS = pools["ps"].tile([8, w], F32, name="psS")
        nc.tensor.matmul(
            psS[:], BONES8, eq8[:, :, l0 : l0 + w],
            start=True, stop=True, perf_mode=DR,
        )
        rt = rtpool.tile([8, w], F32, name="rt")
        rt8 = rtpool.tile([8, w], FP8, name="rt8")
        nc.vector.reciprocal_approx_fast(rt[:], psS[:])
        nc.gpsimd.tensor_copy(rt8[:], rt[:])
        rtb_tiles[a] = rt8

    def qstages(t):
        if 0 <= t - 2 < NLW:
            qwork(t - 2)
        if 0 <= t - 3 < NLW:
            qsum(t - 3)
        if 0 <= t - 4 < NLW:
            qnorm(t - 4, 0)
        if 0 <= t - 5 < NLW:
            qnorm(t - 5, 1)

    def qnorm(a, c):
        # broadcast 1/S to all head partitions (PE bf16), then normalize
        # eq8 in place on DVE reading the PSUM broadcast directly
        w = min(LW, L - a * LW)
        l0 = a * LW
        rt8 = rtb_tiles[a] if c == 0 else rtb_tiles.pop(a)
        prb = pools["prb"].tile([128, w], F32, name="prb")
        nc.tensor.matmul(prb[:], IND8F[:, c, :], rt8[:],
                         start=True, stop=True)
        nc.vector.tensor_tensor(
            eq8[:, c, l0 : l0 + w], eq8[:, c, l0 : l0 + w], prb[:], op=MULT
        )

    # ================= pass 1: k/v proj + Gram with interleaved q =================
    xk_t = xv_t = None
    consts_loaded = False
    for a in range(NLW):
        w = min(LW, L - a * LW)
        l0 = a * LW
        if a % 4 == 0:
            # 2048-wide loads (4 chunks worth) to amortize per-DMA dispatch;
            # the first batch's k/v loads are split so chunk-0 compute only
            # waits on a 512-wide slice
            wd = min(4 * LW, L - l0)
            if not consts_loaded:
                consts_loaded = True
                nc.sync.dma_start(c8[:], ins["c8"][:])
                nc.sync.dma_start(cf[:], ins["cf"][:])
                nc.sync.dma_start(cb[:], ins["cb"][:])
            xk_t = kvpool.tile([128, 2, wd], FP8, name="xk")
            xv_t = kvpool.tile([128, 2, wd], FP8, name="xv")
            for s in range(2):
                if a == 0:
                    nc.sync.dma_start(xk_t[:, s, 0:512], xk_ap[128 * s : 128 * (s + 1), 0:512])
                    nc.sync.dma_start(xv_t[:, s, 0:512], xv_ap[128 * s : 128 * (s + 1), 0:512])
                    nc.sync.dma_start(xk_t[:, s, 512:wd], xk_ap[128 * s : 128 * (s + 1), 512:wd])
                    nc.sync.dma_start(xv_t[:, s, 512:wd], xv_ap[128 * s : 128 * (s + 1), 512:wd])
                else:
                    nc.sync.dma_start(xk_t[:, s, :], xk_ap[128 * s : 128 * (s + 1), l0 : l0 + wd])
                    nc.sync.dma_start(xv_t[:, s, :], xv_ap[128 * s : 128 * (s + 1), l0 : l0 + wd])
            for s in range(2):
                nc.sync.dma_start(
                    xq8_sb[:, s, l0 : l0 + wd], xq8_ap[128 * s : 128 * (s + 1), l0 : l0 + wd]
                )
                nc.sync.dma_start(
                    xqb_sb[:, s, l0 : l0 + wd], xqb_ap[128 * s : 128 * (s + 1), l0 : l0 + wd]
                )
        off = 512 * (a % 4)
        for jj in range(w // 256):  # one 256-l gram group per psum tile
            g = 2 * a + jj
            pk = pkv.tile([128, 512], F32, name="pkv")
            pv = pkv.tile([128, 512], F32, name="pkv")
            for j in range(2):  # j = s of the group (128-l block)
                o = off + 256 * jj + 128 * j
                nc.tensor.matmul(
                    pk[:, 256 * j : 256 * j + 256],
                    xk_t[:, :, o : o + 128], WK8,
                    start=True, stop=True, perf_mode=DR,
                )
                nc.tensor.matmul(
                    pv[:, 256 * j : 256 * j + 256],
                    xv_t[:, :, o : o + 128], WV8,
                    start=True, stop=True, perf_mode=DR,
                )
            nc.scalar.activation(ksm8[:, :, g, :], pk[:], EXP, bias=M2)
            if g % 4 == 1:
                nc.scalar.copy(v8[:, :, g, 0:256], pv[:])
            else:
                nc.vector.tensor_copy(v8[:, :, g, 0:256], pv[:])
            if g - 2 >= 0:
                gram(g - 2)
        qstages(a)

    for g in range(NG - 2, NG):
        gram(g)

    es_kv.close()  # release pk/pv banks

    # ---------------- boundary: build A^T [kch, cout] (fp8) ----------------
    es_bnd = ExitStack()
    bpsum = es_bnd.enter_context(tc.tile_pool(name="bndp", bufs=2, space="PSUM"))
    rk = [bpool.tile([128, 1], F32, name=f"rk{c}") for c in range(2)]
    ctxs = [bpool.tile([128, 256], F32, name=f"ctxs{c}") for c in range(2)]
    for c in range(2):
        nc.vector.reciprocal(rk[c][:], ctx_ps[c][:, 256:257])
        nc.vector.tensor_scalar_mul(ctxs[c][:], ctx_ps[c][:, 0:256], rk[c][:])
    ctxT_ps = [bpsum.tile([128, 256], F32, name="bnd") for a in range(2)]
    for a in range(2):
        for b in range(2):
            nc.tensor.transpose(
                ctxT_ps[a][:, 128 * b : 128 * b + 128],
                ctxs[b][:, 128 * a : 128 * a + 128],
                IDENT,
            )
    ctxT_sb = [bpool.tile([128, 256], BF16, name=f"ctxTs{a}") for a in range(2)]
    for a in range(2):
        nc.vector.memset(ctxT_sb[a][:], 0.0)
    for h in range(HEADS):
        a = h // 4
        p = 32 * (h % 4)
        nc.vector.tensor_scalar_add(
            ctxT_sb[a][p : p + 32, 32 * h : 32 * h + 32],
            ctxT_ps[a][p : p + 32, 32 * h : 32 * h + 32],
            BV[p : p + 32, a : a + 1],
        )
    for g in range(2):
        at_ps = bpsum.tile([128, 256], F32, name="bnd")
        for a in range(2):
            nc.tensor.matmul(
                at_ps[:],
                ctxT_sb[a][:, 128 * g : 128 * g + 128],
                WR[:, 256 * a : 256 * a + 256],
                start=(a == 0), stop=(a == 1),
            )
        nc.scalar.copy(at8[:, g, :], at_ps[:])
    es_bnd.close()
    es_ctx.close()

    # ========= tail: remaining q stages interleaved with pass-2 output =========
    es_c = ExitStack()
    opool = es_c.enter_context(tc.tile_pool(name="op", bufs=3))
    po_pool = es_c.enter_context(tc.tile_pool(name="po", bufs=4, space="PSUM"))

    def pass2(a):
        wd = min(2 * LW, L - a * LW)
        ld = a * LW
        for c in range(2):
            ob = opool.tile([128, wd], BF16, name="ob")
            for hi, half in enumerate(range(0, wd, LW)):
                w = min(LW, wd - half)
                l0 = ld + half
                po = po_pool.tile([128, w], F32, name="po")
                nc.tensor.matmul(
                    po[:], at8[:, :, 128 * c : 128 * c + 128],
                    eq8[:, :, l0 : l0 + w],
                    start=True, stop=True, perf_mode=DR,
                )
                if (2 * c + hi) % 2 == 0:
                    nc.vector.scalar_tensor_tensor(
                        ob[:, half : half + w], po[:], BR[:, c : c + 1],
                        xqb_sb[:, c, l0 : l0 + w], op0=ADD, op1=ADD,
                    )
                else:
                    nc.scalar.activation(
                        ob[:, half : half + w], po[:],
                        mybir.ActivationFunctionType.Identity, bias=BR[:, c : c + 1],
                    )
                    nc.gpsimd.tensor_tensor(
                        ob[:, half : half + w], ob[:, half : half + w],
                        xqb_sb[:, c, l0 : l0 + w], op=ADD,
                    )
            nc.sync.dma_start(out_ap[128 * c : 128 * c + 128, ld : ld + wd], ob[:])

    p2 = 0  # next pass-2 pair start chunk
    for t in range(NLW, NLW + 6):
        qstages(t)
        # pair (p2, p2+1) ready once qnorm(p2+1, 1) has been emitted (t-5)
        while p2 < NLW and min(p2 + 1, NLW - 1) <= t - 5:
            pass2(p2)
            p2 += 2
    es_c.close()
    es_a.close()
    es.close()


def _build_consts(Wq, bq, Wk, bk, Wv, bv, Wr, br):
    bf = ml_dtypes.bfloat16
    f8 = ml_dtypes.float8_e4m3

    def packT8(W):  # [cout, cin] -> [128, 2, 256]: [p, s, o] = W[o, s*128+p]
        t = np.ascontiguousarray(np.asarray(W, np.float32).T)  # [cin, cout]
        return np.stack([t[0:128, :], t[128:256, :]], axis=1)

    ch = np.arange(256)
    bones_full = (ch[:, None] // 32 == np.arange(8)[None, :]).astype(np.float32)  # [256, 8]
    bones8 = np.stack([bones_full[0:128, :], bones_full[128:256, :]], axis=1)  # [128,2,8]
    ind8f = np.zeros((128, 2, 128), np.float32)   # [h, c, j] = head indicator
    for c in range(2):
        for j in range(128):
            ind8f[(c * 128 + j) // 32, c, j] = 1.0
    c8 = np.concatenate(
        [packT8(Wq), packT8(Wk), packT8(Wv), bones8,
         np.zeros((128, 2, 8), np.float32), ind8f], axis=2
    ).astype(f8).reshape(128, 2 * C8_COLS)

    def packT(Wt):  # [cout, cin] -> [128, 512], col block k = W.T[128k:128k+128, :]
        t = np.ascontiguousarray(np.asarray(Wt, np.float32).T)
        return np.concatenate([t[0:128, :], t[128:256, :]], axis=1)

    cb = packT(Wr).astype(bf)
    assert cb.shape == (128, CB_COLS), cb.shape

    def two(v):
        return np.stack([v[0:128], v[128:256]], axis=1).astype(np.float32)

    cf = np.concatenate(
        [two(np.asarray(bq)) - 2.0, two(np.asarray(bv)), two(np.asarray(br)),
         np.eye(128, dtype=np.float32),
         np.full((128, 1), -2.0, np.float32)], axis=1
    ).astype(np.float32)
    assert cf.shape == (128, CF_COLS), cf.shape
    return {"c8": c8, "cb": cb, "cf": cf}


_NC = None


def _build():
    nc = bacc.Bacc("TRN2", target_bir_lowering=False)
    ins = {}
    ins["xq8"] = nc.dram_tensor("xq8", [CIN, L], FP8, kind="ExternalInput").ap()
    ins["xqb"] = nc.dram_tensor("xqb", [CIN, L], BF16, kind="ExternalInput").ap()
    ins["xk8"] = nc.dram_tensor("xk8", [CIN, L], FP8, kind="ExternalInput").ap()
    ins["xv8"] = nc.dram_tensor("xv8", [CIN, L], FP8, kind="ExternalInput").ap()
    ins["c8"] = nc.dram_tensor("c8", [128, 2 * C8_COLS], FP8, kind="ExternalInput").ap()
    ins["cb"] = nc.dram_tensor("cb", [128, CB_COLS], BF16, kind="ExternalInput").ap()
    ins["cf"] = nc.dram_tensor("cf", [128, CF_COLS], F32, kind="ExternalInput").ap()
    out_ap = nc.dram_tensor("out", [CIN, L], BF16, kind="ExternalOutput").ap()
    with tile.TileContext(nc) as tc:
        _emit(tc, ins, out_ap)
    nc.compile()
    return nc


def get_nc():
    global _NC
    if _NC is None:
        _NC = _build()
    return _NC


def make_in_maps(inputs):
    bf = ml_dtypes.bfloat16
    f8 = ml_dtypes.float8_e4m3
    consts = _build_consts(
        inputs["Wq"], inputs["bq"], inputs["Wk"], inputs["bk"],
        inputs["Wv"], inputs["bv"], inputs["Wr"], inputs["br"],
    )
    qf = np.ascontiguousarray(np.asarray(inputs["query_feature"], np.float32)).reshape(N, CIN, L)
    kf = np.asarray(inputs["key_feature"], np.float32).reshape(N, CIN, L)
    vf = np.asarray(inputs["value_feature"], np.float32).reshape(N, CIN, L)
    return [
        {"xq8": np.ascontiguousarray(qf[i].astype(f8)),
         "xqb": np.ascontiguousarray(qf[i].astype(bf)),
         "xk8": np.ascontiguousarray(kf[i].astype(f8)),
         "xv8": np.ascontiguousarray(vf[i].astype(f8)),
         **consts}
        for i in range(N_CORES)
    ]


def kernel(query_feature, key_feature, value_feature,
           Wq, bq, Wk, bk, Wv, bv, Wr, br):
    nc = get_nc()
    in_maps = make_in_maps(dict(
        query_feature=query_feature, key_feature=key_feature,
        value_feature=value_feature, Wq=Wq, bq=bq, Wk=Wk, bk=bk,
        Wv=Wv, bv=bv, Wr=Wr, br=br,
    ))
    res = run_bass_kernel_spmd(nc, in_maps, core_ids=list(range(N_CORES)))
    out = np.stack([res.results[i]["out"].astype(np.float32) for i in range(N_CORES)])
    return out.reshape(N, CIN, H_IMG, W_IMG)
